# revision 22
# baseline (speedup 1.0000x reference)
"""Causal self-attention (B=2, T=2048, D=2048, 16 heads) on 8 trn2 cores.

Sharding: tensor-parallel over heads - 2 heads per core. Each core computes
q/k/v projections for its 2 heads (column-parallel), causal attention per
head, and a partial output projection (row-parallel). Host sums the 8
partial outputs.

v2 structure (PE-continuity focused, bf16):
  Phase P (projections): stream x token-chunks (512 tokens) ONCE; per chunk
    run all 6 matmul groups (2 heads x q/k/v, 16 kt each) into 6 dedicated
    PSUM banks; PSUM->SBUF casts alternate Scalar/Vector engines. v token
    tiles are PE-transposed one chunk behind, interleaved into the stream.
  Phase A (attention): chunk-pairs (both heads, same (b,ic)) with 1-step
    S-matmul lookahead so exp latency (Scalar) never stalls the PE long.
    S banks rotate x4 (shared with the denominator matmul); PV output banks
    rotate x3; softmax normalize (den copy, reciprocal, partition-broadcast,
    multiply) trails off the critical path.
  Phase O (out projection): tt-outer / mc-inner; outT tile is the stationary
    operand, streaming 4x512 wo columns into 4 PSUM banks; PSUM->SBUF
    copies alternate engines; y emitted bf16 (host sums partials in f32).
"""

import math
from contextlib import ExitStack

import numpy as np
import ml_dtypes

import concourse.bass as bass
import concourse.mybir as mybir
import concourse.tile as tile
from concourse import bacc
from concourse.bass_utils import run_bass_kernel_spmd
from concourse.masks import make_identity

P = 128
D_MODEL = 2048
NUM_HEADS = 16
D = 128            # head dim
B, T = 2, 2048
BT = B * T         # 4096
NCORES = 8
HPC = NUM_HEADS // NCORES   # 2 heads per core
KD = D_MODEL // P           # 16 d_model tiles
TJ = T // P                 # 16 key tiles per batch
IC = 512                    # query chunk width
NI = T // IC                # 4 query chunks per batch
TCH = BT // IC              # 8 token chunks for projections
MC = D_MODEL // IC          # 4 outproj column chunks
TT = BT // P                # 32 token tiles

F32 = mybir.dt.float32
BF16 = mybir.dt.bfloat16

_DT = {"f32": mybir.dt.float32, "bf16": mybir.dt.bfloat16}
_NP = {"f32": np.float32, "bf16": ml_dtypes.bfloat16}

# dtype knobs kept for experimentation; bf16 everywhere is validated.
CFG_FAST = dict(x="bf16", w="bf16", s="bf16", pt="bf16", v="bf16", o="bf16",
                wo="bf16", y="bf16")
CFG_SAFE = dict(CFG_FAST)
CFG_F32R = dict(CFG_FAST)


def _emit(tc, cfg, xT, wqT, wkT, wvT, woT, y):
    nc = tc.nc
    x_dt = _DT[cfg["x"]]
    w_dt = _DT[cfg["w"]]
    s_dt = _DT[cfg["s"]]
    pt_dt = _DT[cfg["pt"]]
    v_dt = _DT[cfg["v"]]
    o_dt = _DT[cfg["o"]]
    wo_dt = _DT[cfg["wo"]]
    y_dt = _DT[cfg["y"]]

    with ExitStack() as ctx:
        consts = ctx.enter_context(tc.tile_pool(name="consts", bufs=1))
        wpool = ctx.enter_context(tc.tile_pool(name="wpool", bufs=1))
        xpool = ctx.enter_context(tc.tile_pool(name="xpool", bufs=3))
        arrs = ctx.enter_context(tc.tile_pool(name="arrs", bufs=1))
        vtpool = ctx.enter_context(tc.tile_pool(name="vtpool", bufs=2))
        ptpool = ctx.enter_context(tc.tile_pool(name="ptpool", bufs=8))
        accpool = ctx.enter_context(tc.tile_pool(name="accpool", bufs=2))
        smalls = ctx.enter_context(tc.tile_pool(name="smalls", bufs=2))
        ypool = ctx.enter_context(tc.tile_pool(name="ypool", bufs=5))
        psum = ctx.enter_context(tc.tile_pool(name="psum", bufs=1,
                                              space="PSUM"))

        ident = consts.tile([P, P], v_dt, tag="ident", name="ident")
        make_identity(nc, ident)
        ones_col = consts.tile([P, 1], pt_dt, tag="ones", name="ones")
        nc.vector.memset(ones_col, 1.0)

        # tri_mask[p, i] = 1.0 if i >= p else 0 (upper triangular keep)
        tri_mask = consts.tile([P, P], pt_dt, tag="trimask", name="trimask")
        nc.gpsimd.memset(tri_mask, 0.0)
        nc.gpsimd.affine_select(
            out=tri_mask, in_=tri_mask, compare_op=mybir.AluOpType.is_gt,
            fill=1.0, base=0, pattern=[[-1, P]], channel_multiplier=1,
        )

        xT3 = xT.rearrange("(ko p) t -> p ko t", p=P)
        # weights arrive host-pre-arranged as [p, h, ko, o] (contiguous per
        # partition -> 1 DMA descriptor per partition, single trigger)
        w4 = {
            "q": wqT.rearrange("p (h ko o) -> p h ko o", h=HPC, ko=KD),
            "k": wkT.rearrange("p (h ko o) -> p h ko o", h=HPC, ko=KD),
            "v": wvT.rearrange("p (h ko o) -> p h ko o", h=HPC, ko=KD),
        }
        woT3 = woT.rearrange("(h p) m -> h p m", p=P)

        # ---- weight DMAs, ordered so the first matmul group unblocks fast:
        # wq0 first, then x chunk 0 (emitted below), then the rest.
        w_sb = {}
        for h in range(HPC):
            for nm in ("q", "k", "v"):
                w_sb[(h, nm)] = wpool.tile([P, KD, D], w_dt, tag=f"w{nm}{h}",
                                           name=f"w{nm}{h}")

        def dma_w(h, nm):
            nc.sync.dma_start(w_sb[(h, nm)], w4[nm][:, h])

        dma_w(0, "q")

        # ---- phase P: projections + v transposes ----
        qT = [arrs.tile([P, BT], s_dt, tag=f"qT{h}", name=f"qT{h}") for h in range(HPC)]
        kT = [arrs.tile([P, BT], s_dt, tag=f"kT{h}", name=f"kT{h}") for h in range(HPC)]
        v_sb = [arrs.tile([P, B, TJ, D], v_dt, tag=f"v{h}", name=f"v{h}")
                for h in range(HPC)]
        outT = [arrs.tile([P, BT], o_dt, tag=f"outT{h}", name=f"outT{h}") for h in range(HPC)]

        GROUPS = [(h, nm) for h in range(HPC) for nm in ("q", "k", "v")]
        scale = 1.0 / math.sqrt(D)

        # PSUM plan (8 banks of [128,512]f32):
        #   t0,t1   projection accumulators (rotating per group)
        #   t2,t3   unified short-lived rotation: S tiles, den rows, v-transp
        #   t4..t7  PV accumulators (pair parity picks a disjoint bank pair)
        g_cnt = [0]
        u_rot = [0]

        def u_tile(shape, dt=F32):
            t = psum.tile(shape, dt, tag=f"t{2 + u_rot[0] % 2}",
                          name=f"u{u_rot[0] % 2}")
            u_rot[0] += 1
            return t

        pend_vt = []

        def emit_transposes(items):
            # 8 transposes (2 heads x 4 token tiles) for one chunk
            for h, vt, tch in items:
                for sub in range(4):
                    tok = tch * 4 + sub          # global token tile
                    b, jt = divmod(tok, TJ)
                    pst = u_tile([P, P], v_dt)
                    nc.tensor.transpose(pst, vt[:, sub * P:(sub + 1) * P],
                                        ident)
                    if sub % 2:
                        nc.scalar.copy(v_sb[h][:, b, jt], pst)
                    else:
                        nc.vector.tensor_copy(v_sb[h][:, b, jt], pst)

        def dma_x(tch, lo=0, hi=KD, tiles=None):
            # one tile per kt so each matmul depends only on its own DMA
            if tiles is None:
                tiles = []
            for kt in range(lo, hi):
                xk = xpool.tile([P, IC], x_dt, tag=f"x{kt}", name=f"x{kt}")
                # split triggers across engines at startup (sync serializes
                # DMA issues at ~645ns each; gpsimd is idle then)
                if tch < 2:
                    eng = (nc.sync, nc.gpsimd, nc.scalar)[kt % 3]
                else:
                    eng = nc.sync
                eng.dma_start(xk, xT3[:, kt, tch * IC:(tch + 1) * IC])
                tiles.append(xk)
            return tiles

        xts = [dma_x(0)]
        for h in range(HPC):
            for nm in ("q", "k", "v"):
                if (h, nm) != (0, "q"):
                    dma_w(h, nm)
        xts.append(dma_x(1))
        wo_sb = []
        for h in range(HPC):
            wt = wpool.tile([P, D_MODEL], wo_dt, tag=f"wo{h}", name=f"wo{h}")
            nc.sync.dma_start(wt, woT3[h])
            wo_sb.append(wt)

        # deferred denominator+normalize of the previous chunk-pair, emitted
        # inside the next pair's stream so the PE never waits on the softmax
        # accumulation chain.
        pend_norm = [None]

        def flush_norm():
            if pend_norm[0] is None:
                return
            p_acc, p_o, p_isl = pend_norm[0]
            pend_norm[0] = None
            for h in range(HPC):
                ps_d = u_tile([1, IC])
                nc.tensor.matmul(ps_d, ones_col, p_acc[h],
                                 start=True, stop=True,
                                 skip_group_check=True)
                den_sb = smalls.tile([1, IC], F32, tag=f"den{h}",
                                     name=f"den{h}")
                nc.vector.tensor_copy(den_sb, ps_d)
                rb1 = smalls.tile([1, IC], F32, tag=f"rb1{h}",
                                  name=f"rb1{h}")
                nc.vector.reciprocal_approx_fast(out=rb1, in_=den_sb)
                bc = smalls.tile([P, IC], F32, tag=f"bc{h}", name=f"bc{h}")
                nc.gpsimd.partition_broadcast(bc, rb1)
                nc.vector.tensor_tensor(
                    outT[h][:, p_isl], p_o[h], bc, mybir.AluOpType.mult)

        def pair_gen(pi, b, ic):
            """Attention for both heads of query chunk (b, ic); yields at
            step boundaries so the driver can interleave projection work."""
            nj = ic * 4 + 4
            isl = slice(b * T + ic * IC, b * T + (ic + 1) * IC)
            base = 4 + 2 * (pi % 2)
            ps_o = [psum.tile([P, IC], F32, tag=f"t{base + h}",
                              name=f"o{base + h}") for h in range(HPC)]
            pt_acc = [accpool.tile([P, IC], pt_dt, tag=f"acc{h}",
                                   name=f"acc{h}") for h in range(HPC)]
            ps_s = [[None] * nj for _ in range(HPC)]
            pts = [[None] * nj for _ in range(HPC)]

            def lo_of(jt):
                return max(jt - ic * 4, 0) * P

            def emit_S(h, jt):
                lo = lo_of(jt)
                ps = u_tile([P, IC])
                nc.tensor.matmul(
                    ps[:, lo:],
                    kT[h][:, b * T + jt * P: b * T + (jt + 1) * P],
                    qT[h][:, b * T + ic * IC + lo: b * T + (ic + 1) * IC],
                    start=True, stop=True)
                ps_s[h][jt] = ps

            def emit_exp(h, jt):
                lo = lo_of(jt)
                m = jt - ic * 4
                pt = ptpool.tile([P, IC], pt_dt, tag="pt", name="pt")
                nc.scalar.activation(
                    pt[:, lo:], ps_s[h][jt][:, lo:],
                    mybir.ActivationFunctionType.Exp, scale=scale)
                if m >= 0:
                    nc.vector.tensor_tensor(
                        pt[:, lo:lo + P], pt[:, lo:lo + P],
                        tri_mask, mybir.AluOpType.mult)
                if jt == 0:
                    nc.vector.tensor_copy(pt_acc[h], pt)
                else:
                    nc.vector.tensor_tensor(
                        pt_acc[h][:, lo:], pt_acc[h][:, lo:],
                        pt[:, lo:], mybir.AluOpType.add)
                pts[h][jt] = pt

            def emit_PV(h, jt):
                lo = lo_of(jt)
                nc.tensor.matmul(
                    ps_o[h][:, lo:], v_sb[h][:, b, jt], pts[h][jt][:, lo:],
                    start=(jt == 0), stop=(jt == nj - 1),
                    skip_group_check=True)

            for h in range(HPC):
                emit_S(h, 0)
            for h in range(HPC):
                emit_exp(h, 0)
            yield
            for jt in range(1, nj):
                for h in range(HPC):
                    emit_S(h, jt)
                for h in range(HPC):
                    emit_exp(h, jt)
                if jt == 1:
                    flush_norm()   # previous pair's den + normalize
                for h in range(HPC):
                    emit_PV(h, jt - 1)
                yield
            for h in range(HPC):
                emit_PV(h, nj - 1)
            pend_norm[0] = (pt_acc, ps_o, isl)

        def run_steps(gen, n):
            if gen is None:
                return None
            for _ in range(n):
                try:
                    next(gen)
                except StopIteration:
                    return None
            return gen

        # ---- merged phase: projections with attention pairs interleaved ----
        PAIRS = [(b, ic) for b in range(B) for ic in range(NI)]
        cur = None
        for tch in range(TCH):
            xt = xts[tch]
            tsl = slice(tch * IC, (tch + 1) * IC)
            vt_items = []
            pi = tch - 1
            if 0 <= pi < 6:
                assert cur is None
                cur = pair_gen(pi, *PAIRS[pi])
                units = (pi % 4) * 4 + 4 + 1
                k = -(-units // 6)          # steps per interleave slot
            else:
                k = 0
            for gi, (h, nm) in enumerate(GROUPS):
                ps = psum.tile([P, IC], F32, tag=f"t{g_cnt[0] % 2}",
                               name=f"p{g_cnt[0] % 2}")
                g_cnt[0] += 1
                for kt in range(KD):
                    nc.tensor.matmul(ps, w_sb[(h, nm)][:, kt], xt[kt],
                                     start=(kt == 0), stop=(kt == KD - 1))
                if nm == "q":
                    dst = qT[h][:, tsl]
                elif nm == "k":
                    dst = kT[h][:, tsl]
                else:
                    dst = vtpool.tile([P, IC], v_dt, tag=f"vt{h}",
                                      name=f"vt{h}")
                    vt_items.append((h, dst, tch))
                    dst = dst[:, :]
                if gi % 2 == 0:
                    nc.vector.tensor_copy(dst, ps)
                else:
                    nc.scalar.copy(dst, ps)
                # interleave: last chunk's transposes after group 0,
                # attention steps after the other groups; stagger the
                # chunk+2 x prefetch so it never starves this chunk's DMAs
                if gi == 0:
                    if pend_vt:
                        emit_transposes(pend_vt)
                        pend_vt = []
                else:
                    cur = run_steps(cur, k)
                if tch + 2 < TCH:
                    if gi == 2:
                        xts.append(dma_x(tch + 2, 0, 8))
                    elif gi == 4:
                        dma_x(tch + 2, 8, KD, xts[tch + 2])
            cur = run_steps(cur, k)          # end-of-chunk slot
            pend_vt = vt_items
        emit_transposes(pend_vt)
        while cur is not None:
            cur = run_steps(cur, 4)

        # ---- phase O: output projection, interleaved into the attention
        # tail (pairs 6, 7 are Scalar-bound; outproj units fill PE idle) ----
        UNITS = [(tt, mc) for tt in range(TT) for mc in range(MC)]
        up = [0]      # unit pointer
        yc = [0]      # psum rotation for units
        # y rows are written per token tile: 4 mc units share one [P, 2048]
        # staging tile and a single DMA (1 descriptor per partition)
        y_stage = [None]

        def emit_unit(inter):
            tt, mc = UNITS[up[0]]
            up[0] += 1
            tag = f"t{yc[0] % 2}" if inter else f"t{yc[0] % 8}"
            yc[0] += 1
            ps_y = psum.tile([P, IC], F32, tag=tag, name="yps")
            for hh in range(HPC):
                nc.tensor.matmul(
                    ps_y, outT[hh][:, tt * P:(tt + 1) * P],
                    wo_sb[hh][:, mc * IC:(mc + 1) * IC],
                    start=(hh == 0), stop=(hh == HPC - 1))
            if mc == 0:
                y_stage[0] = ypool.tile([P, D_MODEL], y_dt, tag="y",
                                        name="ysb")
            y_sb = y_stage[0]
            if inter or yc[0] % 2 == 0:
                nc.vector.tensor_copy(y_sb[:, mc * IC:(mc + 1) * IC], ps_y)
            else:
                nc.scalar.copy(y_sb[:, mc * IC:(mc + 1) * IC], ps_y)
            if mc == MC - 1:
                eng = nc.sync if (tt % 2 == 0 or inter) else nc.gpsimd
                eng.dma_start(y[tt * P:(tt + 1) * P], y_sb)

        for pi in (6, 7):
            cur = pair_gen(pi, *PAIRS[pi])
            ready_tt = 16 if pi == 6 else 24
            step_i = 0
            while cur is not None:
                cur = run_steps(cur, 1)
                step_i += 1
                if (step_i >= 3 and up[0] < len(UNITS)
                        and UNITS[up[0]][0] < ready_tt):
                    emit_unit(True)
        flush_norm()
        while up[0] < len(UNITS):
            emit_unit(False)


def _build(cfg):
    nc = bacc.Bacc("TRN2", target_bir_lowering=False, debug=False,
                   num_devices=NCORES)
    xT = nc.dram_tensor("xT", [D_MODEL, BT], _DT[cfg["x"]],
                        kind="ExternalInput").ap()
    wqT = nc.dram_tensor("wqT", [P, HPC * KD * D], _DT[cfg["w"]],
                         kind="ExternalInput").ap()
    wkT = nc.dram_tensor("wkT", [P, HPC * KD * D], _DT[cfg["w"]],
                         kind="ExternalInput").ap()
    wvT = nc.dram_tensor("wvT", [P, HPC * KD * D], _DT[cfg["w"]],
                         kind="ExternalInput").ap()
    woT = nc.dram_tensor("woT", [HPC * D, D_MODEL], _DT[cfg["wo"]],
                         kind="ExternalInput").ap()
    y = nc.dram_tensor("y", [BT, D_MODEL], _DT[cfg["y"]],
                       kind="ExternalOutput").ap()
    with tile.TileContext(nc) as tc:
        _emit(tc, cfg, xT, wqT, wkT, wvT, woT, y)
    nc.compile()
    return nc


def _prep_inputs(x, Wq, Wk, Wv, Wo, cfg):
    xnp = _NP[cfg["x"]]
    wnp = _NP[cfg["w"]]
    wonp = _NP[cfg["wo"]]
    xT = np.ascontiguousarray(
        np.asarray(x, np.float32).reshape(BT, D_MODEL).T).astype(xnp)
    def w_pre(W, rows):
        # [p, h, ko, o] layout: contiguous per partition for 1-descriptor DMA
        A = np.asarray(W)[rows]                      # [HPC*D, D_MODEL]
        Bv = A.reshape(HPC, D, KD, P).transpose(3, 0, 2, 1)
        return np.ascontiguousarray(
            Bv.reshape(P, HPC * KD * D)).astype(wnp)

    in_maps = []
    for c in range(NCORES):
        rows = slice(c * HPC * D, (c + 1) * HPC * D)
        in_maps.append({
            "xT": xT,
            "wqT": w_pre(Wq, rows),
            "wkT": w_pre(Wk, rows),
            "wvT": w_pre(Wv, rows),
            "woT": np.ascontiguousarray(
                np.asarray(Wo)[:, rows].T).astype(wonp),
        })
    return in_maps


def run(x, Wq, Wk, Wv, Wo, cfg=None, trace=False):
    cfg = cfg or CFG_FAST
    nc = _build(cfg)
    in_maps = _prep_inputs(x, Wq, Wk, Wv, Wo, cfg)
    try:
        res = run_bass_kernel_spmd(nc, in_maps, core_ids=list(range(NCORES)),
                                   trace=trace)
    except Exception:
        res = run_bass_kernel_spmd(nc, in_maps, core_ids=list(range(NCORES)),
                                   trace=trace)
    y = np.zeros((BT, D_MODEL), np.float32)
    for r in res.results:
        y += np.asarray(r["y"], dtype=np.float32)
    return y.reshape(B, T, D_MODEL), res


def kernel(x, Wq, Wk, Wv, Wo):
    y, _ = run(x, Wq, Wk, Wv, Wo)
    return y
